# revision 37
# baseline (speedup 1.0000x reference)
"""Trainium2 Bass kernel for nn_DCConv3dKernelPolynomials.

Computes out[m,n,b,p] = sum_k coeff[m,n,k] * psi_k(position[b,p,:])
where psi_k are the 23 real hydrogen-like wavefunctions (n<=4, l<=2).

Key math: with u=x/r, v=y/r, w=z/r, the angular factors are pure
polynomials, so the device only needs sqrt/reciprocal/exp plus
polynomial arithmetic.  All four radial exponentials come from one
exp:  t=e^{-r/12}; e4=t^3, e3=t^4, e2=t^6, e1=t^12.

Perf design (HBM-write-roofline bound, ~94us/core):
  * fp16 output (rel-err ~5e-4, gate is 2e-2) halves the write wall
    vs f32: 32 MiB/core at ~358 GB/s.
  * fp16 K=23 matmul, packed 4-up into 32-row PE strips via
    tile_position=(32i,0): 4 point-blocks computed concurrently, so
    the PE is never the bottleneck (no HAM-throttle sensitivity).
  * poly transposes packed 4-up into 32-col PE strips, written
    straight into a borrowed 4-bank PSUM tile laid out as poly_t4.
  * PSUM->SBUF fp32->fp16 copies on 4-bank [128,2048] tiles (amortizes
    the per-op bubble), split vector/scalar 26/38 to balance engines.
  * per-mt [128,4096] fp16 stages -> 32 x 1 MiB HWDGE DMAs on the
    sync ring (8 KiB/partition contiguous lines).

Sharding: batch b -> core b (8 cores, 4096 points each).
Point order on core: point q lives at (partition p=q%128, chunk
c=q//128), so transposed chunk c lands at poly_t4 block i=c//8
(partitions 32i..32i+22), cols 128*(c%8)..+128 == natural column q.
"""

import math

import numpy as np

B = 8
PTS = 4096            # points per core
OUTC = INC = 64
MN = OUTC * INC       # 4096
NB = 23               # basis functions
NCORES = 8
NCH = 32              # chunks of 128 points
NMT = MN // 128       # 32 mn tiles
NBLK = 4              # point blocks (PE row strips)
PPB = PTS // NBLK     # 1024 points per block
SCALAR_COPIES = 63    # ACT share of the 120 non-forced stage copies


def _combos():
    combos = []
    for n in range(1, 5):
        for k in range(3):
            for m in range(-3, 4):
                if abs(m) <= k and k < n:
                    combos.append((n, k, m))
    return combos


COMBOS = _combos()
assert len(COMBOS) == NB


def _basis_scales():
    """Per-basis constant c_k so that psi_k = c_k * Rb_dev(n,l) * A_dev(l,m).

    Rb_dev / A_dev are the *unnormalized* tile products computed on device:
      Rb_dev(1,0)=e1, Rb_dev(2,0)=(2-r)e2, Rb_dev(2,1)=r*e2,
      Rb_dev(3,0)=(2r-(2/9)r^2-3)e3   [= -L_2^1(2r/3)e3]
      Rb_dev(3,1)=((8/3)r-(4/9)r^2)e3, Rb_dev(3,2)=r^2*e3,
      Rb_dev(4,0)=(4-3r+r^2/2-r^3/48)e4,
      Rb_dev(4,1)=(5r-(5/4)r^2+r^3/16)e4, Rb_dev(4,2)=(6-r/2)r^2*e4
      A_dev: 1, w, u, v, 3w^2-1, wu, wv, u^2-v^2, uv
    """
    fourpi = 4.0 * math.pi
    K00 = math.sqrt(1.0 / fourpi)
    K10 = math.sqrt(3.0 / fourpi)
    K11 = math.sqrt(3.0 / (2.0 * fourpi))
    K20 = math.sqrt(5.0 / fourpi)
    K21 = math.sqrt(5.0 / (6.0 * fourpi))
    K22 = math.sqrt(5.0 / (24.0 * fourpi))
    s2 = math.sqrt(2.0)

    def norm_r(n, l):
        return math.sqrt(
            (2.0 / n) ** 3 * math.factorial(n - l - 1)
            / (2 * n * math.factorial(n + l))
        )

    f = {
        (1, 0): norm_r(1, 0),
        (2, 0): norm_r(2, 0),
        (2, 1): norm_r(2, 1),
        (3, 0): -norm_r(3, 0),
        (3, 1): norm_r(3, 1),
        (3, 2): norm_r(3, 2) * (4.0 / 9.0),
        (4, 0): norm_r(4, 0),
        (4, 1): norm_r(4, 1),
        (4, 2): norm_r(4, 2) * 0.25,
    }
    a = {
        (0, 0): K00,
        (1, 0): K10,
        (1, 1): -s2 * K11,
        (1, -1): -s2 * K11,
        (2, 0): 0.5 * K20,
        (2, 1): -3.0 * s2 * K21,
        (2, -1): -3.0 * s2 * K21,
        (2, 2): 3.0 * s2 * K22,
        (2, -2): 6.0 * s2 * K22,
    }
    return np.array(
        [f[(n, l)] * a[(l, m)] for (n, l, m) in COMBOS], dtype=np.float64
    )


def poly_host(position):
    """Numpy replica of the device basis recipe (for self-checking)."""
    pos = np.asarray(position, dtype=np.float32)
    x, y, z = pos[..., 0], pos[..., 1], pos[..., 2]
    r2 = x * x + y * y + z * z
    r = np.sqrt(r2)
    ir = 1.0 / r
    u, v, w = x * ir, y * ir, z * ir
    e1, e2, e3, e4 = np.exp(-r), np.exp(-r / 2), np.exp(-r / 3), np.exp(-r / 4)
    rr = r * r
    A = {
        (0, 0): np.ones_like(r),
        (1, 0): w, (1, 1): u, (1, -1): v,
        (2, 0): 3 * w * w - 1, (2, 1): w * u, (2, -1): w * v,
        (2, 2): u * u - v * v, (2, -2): u * v,
    }
    Rb = {
        (1, 0): e1,
        (2, 0): (2 - r) * e2,
        (2, 1): r * e2,
        (3, 0): (2 * r - (2.0 / 9.0) * rr - 3) * e3,
        (3, 1): ((8.0 / 3.0) * r - (4.0 / 9.0) * rr) * e3,
        (3, 2): rr * e3,
        (4, 0): (4 - 3 * r + rr / 2 - rr * r / 48) * e4,
        (4, 1): (5 * r - 1.25 * rr + rr * r / 16) * e4,
        (4, 2): (6 - r / 2) * rr * e4,
    }
    c = _basis_scales()
    return np.stack(
        [
            (c[k] * Rb[(n, l)] * A[(l, m)]).astype(np.float32)
            for k, (n, l, m) in enumerate(COMBOS)
        ],
        axis=-1,
    )


_PROGRAM = None


def _build_program():
    import concourse.bacc as bacc
    import concourse.tile as tile
    from concourse import mybir
    from concourse.bass import ts
    from concourse.masks import make_identity

    f32 = mybir.dt.float32
    f16 = mybir.dt.float16
    AF = mybir.ActivationFunctionType
    ALU = mybir.AluOpType

    nc = bacc.Bacc(trn_type="TRN2")
    pos_d = nc.dram_tensor("position", [128, 96], f32, kind="ExternalInput")
    coefft_d = nc.dram_tensor("coefft4", [128, MN], f16, kind="ExternalInput")
    out_d = nc.dram_tensor("out", [MN, PTS], f16, kind="ExternalOutput")

    with tile.TileContext(nc) as tc:
        with (
            tc.tile_pool(name="const", bufs=1) as const,
            tc.tile_pool(name="pw", bufs=1) as pw,
            tc.tile_pool(name="stage", bufs=4) as stage_pool,
            tc.tile_pool(name="psum", bufs=4, space="PSUM") as psum,
        ):
            # inputs on the HWDGE scalar ring (its preamble drains ~2us
            # before the sync ring's; gpsimd is busy with memsets)
            xyz = const.tile([128, 96], f32, tag="xyz", name="xyz")
            nc.scalar.dma_start(out=xyz[:, 0:48], in_=pos_d[:, 0:48])
            nc.scalar.dma_start(out=xyz[:, 48:96], in_=pos_d[:, 48:96])
            coefft4 = const.tile([128, MN], f16, tag="coefft4", name="coefft4_sb")
            nc.scalar.dma_start(out=coefft4[:], in_=coefft_d[:, :])

            ident = const.tile([128, 128], f16, tag="ident", name="ident")
            make_identity(nc, ident[:])

            # preload the exp ACT table on dummy data before the inputs
            # arrive (exp is the ONLY table function used -- r and 1/r come
            # from a DVE bit-trick rsqrt, so no table reloads ever happen)
            warm = pw.tile([128, 16], f32, tag="warm", name="warm")
            warm2 = pw.tile([128, 16], f32, tag="warm2", name="warm2")
            nc.gpsimd.memset(warm[:], 1.0)
            nc.scalar.activation(warm2[:], warm[:], AF.Exp)

            xyz3 = xyz[:].rearrange("p (c t) -> p c t", t=3)

            def T(tag):
                return pw.tile([128, NCH], f32, tag=tag, name=tag)[:]

            def bcastn(ap2d, n):
                import concourse.bass as bass
                return bass.AP(
                    tensor=ap2d.tensor,
                    offset=ap2d.offset,
                    ap=[ap2d.ap[0], [0, n], ap2d.ap[1]],
                )

            # basis values land in poly16[:, c, k] fp16; k padded to 32.
            # Transposing 4 consecutive chunks at once ([128, 4x32] ->
            # [128, 128]) puts chunk 4b+j's psi_k at PSUM rows 32j+k, so
            # strip j receives its own chunk with no replication; the
            # point->(partition, chunk) permutation is folded into the
            # host-side layout (see _prep_inputs).
            poly16p = const.tile([128, NCH, 32], f16, tag="poly16", name="poly16")
            nc.gpsimd.memset(poly16p[:, :, NB:32], 0.0)
            poly_t4 = const.tile([128, PPB], f16, tag="poly_t4", name="poly_t4")

            (r2, r, ir, rr, lnr2, t1, t2, e2, e3, e4, uu, vv, t20, rb21,
             p30, rb31, rb32, p40, p40b, rb41, rb42) = (
                T(t) for t in (
                    "r2 r ir rr lnr2 t1 t2 e2 e3 e4 uu vv t20 rb21 "
                    "p30 rb31 rb32 p40 p40b rb41 rb42"
                ).split()
            )
            vwu_t = pw.tile([128, 3, NCH], f32, tag="vwu", name="vwu")[:]
            ang5_t = pw.tile([128, 5, NCH], f32, tag="ang5", name="ang5")[:]

            def _H(t2d):
                return t2d[:, 0:16]

            def pw_radial(h):
                """r, 1/r, exps, angulars for chunk half h.  Both halves
                share the SAME temp columns so the scheduler is forced to
                finish h0's chain before starting h1 (keeps h0's transposes
                off the back of an interleaved 2x-long vector queue)."""
                sl = slice(16 * h, 16 * (h + 1))
                H = _H

                x, y, z = (xyz3[:, sl, t] for t in range(3))
                hr2, hr, hir, hrr, hnw = (
                    H(t) for t in (r2, r, ir, rr, lnr2)
                )
                nc.vector.tensor_mul(hr2, x, x)
                tAh, tBh = H(uu), H(vv)   # reuse as scratch pre-uu/vv
                nc.gpsimd.tensor_mul(tAh, y, y)
                nc.gpsimd.tensor_mul(tBh, z, z)
                nc.vector.tensor_add(hr2, hr2, tAh)
                nc.vector.tensor_add(hr2, hr2, tBh)
                # 1/r = rsqrt(r2) via the int32 bit trick + 2 Newton steps
                # (rel err ~5e-6, far below the fp16 output rounding)
                i32 = mybir.dt.int32
                nc.vector.tensor_scalar(
                    hir.bitcast(i32), hr2.bitcast(i32), 1, None,
                    ALU.logical_shift_right,
                )
                nc.vector.tensor_scalar(
                    hir.bitcast(i32), hir.bitcast(i32), 0x5F3759DF, -1,
                    ALU.subtract, ALU.mult,
                )
                for _ in range(2):
                    nc.vector.tensor_mul(hnw, hir, hir)
                    nc.vector.tensor_mul(hnw, hnw, hr2)
                    nc.vector.tensor_scalar(
                        hnw, hnw, -0.5, 1.5, ALU.mult, ALU.add
                    )
                    nc.vector.tensor_mul(hir, hir, hnw)
                nc.vector.tensor_mul(hr, hr2, hir)

                # radial exponentials from one exp: t1=e^{-r/12}
                ht1, ht2, he2, he3, he4 = (H(t) for t in (t1, t2, e2, e3, e4))
                nc.scalar.activation(ht1, hr, AF.Exp, scale=-1.0 / 12.0)
                nc.vector.tensor_mul(ht2, ht1, ht1)
                nc.vector.tensor_mul(he4, ht2, ht1)
                nc.vector.tensor_mul(he3, ht2, ht2)
                nc.vector.tensor_mul(he2, he3, ht2)

                vwu = vwu_t[:, :, 0:16]
                ang5 = ang5_t[:, :, 0:16]
                v, w, u = vwu[:, 0, :], vwu[:, 1, :], vwu[:, 2, :]
                uv, wv, a20, wu, a22 = (ang5[:, i, :] for i in range(5))
                nc.vector.tensor_mul(v, y, hir)
                nc.vector.tensor_mul(w, z, hir)
                nc.vector.tensor_mul(u, x, hir)
                nc.vector.tensor_mul(hrr, hr, hr)

                huu, hvv = H(uu), H(vv)
                nc.gpsimd.tensor_mul(a20, w, w)
                nc.gpsimd.tensor_scalar(a20, a20, 3.0, -1.0, ALU.mult, ALU.add)
                nc.gpsimd.tensor_mul(huu, u, u)
                nc.gpsimd.tensor_mul(hvv, v, v)
                nc.gpsimd.tensor_sub(a22, huu, hvv)
                nc.gpsimd.tensor_mul(uv, u, v)
                nc.gpsimd.tensor_mul(wu, w, u)
                nc.gpsimd.tensor_mul(wv, w, v)

            def _poly_views(h):
                sl = slice(16 * h, 16 * (h + 1))
                H = _H
                poly_s = poly16p[:, sl, 0:NB]
                slot = [poly16p[:, sl, k] for k in range(NB)]
                poly_kc = poly_s.rearrange("p c k -> p k c")
                return H, slot, poly_kc

            def pw_poly_a(h):
                """Basis products, slots 0..13."""
                H, slot, poly_kc = _poly_views(h)
                hr, hrr = H(r), H(rr)
                he2, he3 = H(e2), H(e3)
                vwu = vwu_t[:, :, 0:16]
                ang5 = ang5_t[:, :, 0:16]

                nc.vector.tensor_mul(slot[0], he2, he2)          # e1
                ht20 = H(t20)
                nc.vector.tensor_scalar(ht20, hr, -1.0, 2.0, ALU.mult, ALU.add)
                nc.vector.tensor_mul(slot[1], ht20, he2)
                hrb21 = H(rb21)
                nc.vector.tensor_mul(hrb21, hr, he2)
                nc.vector.tensor_mul(poly_kc[:, 2:5, :], bcastn(hrb21, 3), vwu)
                hp30 = H(p30)
                nc.vector.tensor_scalar(hp30, hrr, 2.0 / 9.0, 3.0, ALU.mult, ALU.add)
                nc.vector.scalar_tensor_tensor(
                    hp30, hr, 2.0, hp30, ALU.mult, ALU.subtract
                )
                nc.vector.tensor_mul(slot[5], hp30, he3)
                hrb31 = H(rb31)
                nc.vector.tensor_scalar(
                    hrb31, hr, -4.0 / 9.0, 8.0 / 3.0, ALU.mult, ALU.add
                )
                nc.vector.tensor_mul(hrb31, hrb31, hr)
                nc.vector.tensor_mul(hrb31, hrb31, he3)
                nc.vector.tensor_mul(poly_kc[:, 6:9, :], bcastn(hrb31, 3), vwu)
                hrb32 = H(rb32)
                nc.vector.tensor_mul(hrb32, hrr, he3)
                nc.vector.tensor_mul(poly_kc[:, 9:14, :], bcastn(hrb32, 5), ang5)

            def pw_poly_b(h):
                """Basis products, slots 14..22."""
                H, slot, poly_kc = _poly_views(h)
                hr, hrr = H(r), H(rr)
                he4 = H(e4)
                vwu = vwu_t[:, :, 0:16]
                ang5 = ang5_t[:, :, 0:16]
                hp40, hp40b = H(p40), H(p40b)
                nc.vector.tensor_scalar(hp40, hr, -1.0 / 48.0, 0.5, ALU.mult, ALU.add)
                nc.vector.tensor_mul(hp40, hp40, hrr)
                nc.vector.tensor_scalar(hp40b, hr, -3.0, 4.0, ALU.mult, ALU.add)
                nc.vector.tensor_add(hp40, hp40, hp40b)
                nc.vector.tensor_mul(slot[14], hp40, he4)
                hrb41 = H(rb41)
                nc.vector.tensor_scalar(hrb41, hr, 1.0 / 16.0, -1.25, ALU.mult, ALU.add)
                nc.vector.tensor_mul(hrb41, hrb41, hr)
                nc.vector.tensor_scalar(hrb41, hrb41, 5.0, None, ALU.add)
                nc.vector.tensor_mul(hrb41, hrb41, hr)
                nc.vector.tensor_mul(hrb41, hrb41, he4)
                nc.vector.tensor_mul(poly_kc[:, 15:18, :], bcastn(hrb41, 3), vwu)
                hrb42 = H(rb42)
                nc.vector.tensor_scalar(hrb42, hr, -0.5, 6.0, ALU.mult, ALU.add)
                nc.vector.tensor_mul(hrb42, hrb42, hrr)
                nc.vector.tensor_mul(hrb42, hrb42, he4)
                nc.vector.tensor_mul(poly_kc[:, 18:23, :], bcastn(hrb42, 5), ang5)

            # ---- emission helpers ----------------------------------------
            def transpose_half(h, drain_eng):
                tr = psum.tile([128, 1024], f32, tag="mm", name="tr_ps")
                trv = tr[:].bitcast(f16)
                for bb in range(4):
                    b = 4 * h + bb
                    nc.tensor.transpose(
                        trv[:, 128 * bb:128 * (bb + 1)],
                        poly16p[:, 4 * b:4 * (b + 1), :],
                        ident[:],
                    )
                drain_eng(poly_t4[:, ts(h, 512)], trv[:, 0:512])

            # per (mt, h): 4 strip-concurrent MMs into two 2-bank psum
            # tiles (strips 0,1 -> A; 2,3 -> B), drained by both engines in
            # parallel into stage_ap; psum bank j == stage cols 2048h+512j
            # == points 2048h+512j.. (identity order via the host layout).
            copy_state = [0, 0]   # [idx, scalar_cum]

            def stage_copy(dst, src, paired=None):
                if paired is not None:
                    if paired == 0:
                        nc.scalar.copy(dst, src)
                    else:
                        nc.vector.tensor_copy(dst, src)
                    return
                idx, cum = copy_state
                s_next = ((idx + 1) * SCALAR_COPIES) // 120
                copy_state[0] = idx + 1
                copy_state[1] = s_next
                if s_next > cum:
                    nc.scalar.copy(dst, src)
                else:
                    nc.vector.tensor_copy(dst, src)

            def mm_group(mt, h, stage_ap, paired=False):
                for q in range(2):
                    ps = psum.tile([128, 1024], f32, tag="mm", name="mmps")
                    for jj in range(2):
                        j = 2 * q + jj
                        nc.tensor.matmul(
                            ps[:, ts(jj, 512)],
                            lhsT=coefft4[32 * j:32 * j + NB, ts(mt, 128)],
                            rhs=poly_t4[32 * j:32 * j + NB, ts(h, 512)],
                            start=True,
                            stop=True,
                            tile_position=(32 * j, 0),
                        )
                    stage_copy(
                        stage_ap[:, ts(q, 1024)], ps[:], q if paired else None
                    )

            def half_stage(mt, h, paired=False):
                stage = stage_pool.tile(
                    [128, 2048], f16, tag="stage_h", name="stage_h", bufs=3
                )
                mm_group(mt, h, stage, paired)
                nc.sync.dma_start(
                    out=out_d[ts(mt, 128), ts(h, 2048)], in_=stage[:]
                )

            # ---- emission order: h0 chain; 4 early h0 groups run on the
            # PE between the two transpose batches, as 512 KiB half-stage
            # DMAs; from mt=4 on, full-row 1 MiB stages (8 KiB HBM runs).
            pw_radial(0)
            pw_poly_a(0)
            pw_poly_b(0)
            transpose_half(0, nc.scalar.copy)
            pw_radial(1)
            pw_poly_a(1)
            for mt in range(4):
                half_stage(mt, 0, paired=True)
            pw_poly_b(1)
            transpose_half(1, nc.vector.tensor_copy)
            for mt in range(4):
                half_stage(mt, 1)
            for mt2 in range(2, NMT // 2 - 1):
                stage = stage_pool.tile(
                    [128, 2, PTS], f16, tag="stage", name="stage", bufs=3
                )
                for q in range(2):
                    mt = 2 * mt2 + q
                    mm_group(mt, 0, stage_ap=stage[:, q, 0:2048])
                    mm_group(mt, 1, stage_ap=stage[:, q, 2048:4096])
                dest = out_d[2 * mt2 * 128:(2 * mt2 + 2) * 128, :].rearrange(
                    "(q p) j -> p q j", p=128
                )
                nc.sync.dma_start(out=dest, in_=stage[:, :, :])
            # penultimate mt as one 1 MiB stage, last as two halves -> tail
            stage1 = stage_pool.tile(
                [128, PTS], f16, tag="stage1", name="stage1", bufs=1
            )
            mm_group(NMT - 2, 0, stage_ap=stage1[:, 0:2048])
            mm_group(NMT - 2, 1, stage_ap=stage1[:, 2048:4096])
            nc.sync.dma_start(out=out_d[ts(NMT - 2, 128), :], in_=stage1[:])
            half_stage(NMT - 1, 0)
            half_stage(NMT - 1, 1)

    nc.finalize()
    return nc


def _get_program():
    global _PROGRAM
    if _PROGRAM is None:
        _PROGRAM = _build_program()
    return _PROGRAM


def _prep_inputs(position, coefficients):
    pos = np.asarray(position, dtype=np.float32)
    coeff = np.asarray(coefficients, dtype=np.float32)
    assert pos.shape == (B, PTS, 3) and coeff.shape == (OUTC, INC, NB)
    c = _basis_scales().astype(np.float32)
    C = (coeff * c).reshape(MN, NB).T.astype(np.float16)  # [23, 4096]
    coefft4 = np.zeros((128, MN), dtype=np.float16)
    for i in range(NBLK):
        coefft4[32 * i:32 * i + NB] = C
    # point q = 2048h + 512j + 128t + p  ->  (partition p, chunk 16h+4t+j)
    # so that the batch-of-4 transpose + strip-j matmul leaves the output
    # columns in natural point order.
    cs = np.arange(NCH)
    h, t, j = cs // 16, (cs % 16) // 4, cs % 4
    q0 = 2048 * h + 512 * j + 128 * t
    idx = q0[None, :] + np.arange(128)[:, None]   # [128 p, 32 c] -> q
    return [
        {
            "position": np.ascontiguousarray(
                pos[b][idx].reshape(128, 96)
            ),
            "coefft4": coefft4,
        }
        for b in range(B)
    ]


def _assemble(results):
    return np.stack(
        [
            np.asarray(r["out"]).astype(np.float32).reshape(OUTC, INC, PTS)
            for r in results
        ],
        axis=2,
    )


def kernel(position, coefficients):
    from concourse import bass_utils

    nc = _get_program()
    in_maps = _prep_inputs(position, coefficients)
    res = bass_utils.run_bass_kernel_spmd(nc, in_maps, core_ids=list(range(NCORES)))
    return _assemble(res.results)


def kernel_traced(position, coefficients, trace_cores=None):
    """Like kernel() but captures an NTFF trace; returns (out, results)."""
    from concourse import bass_utils

    nc = _get_program()
    in_maps = _prep_inputs(position, coefficients)
    res = bass_utils.run_bass_kernel_spmd(
        nc,
        in_maps,
        core_ids=list(range(NCORES)),
        trace=True,
        trace_cores=trace_cores,
    )
    return _assemble(res.results), res


# revision 38
# speedup vs baseline: 1.1911x; 1.1911x over previous
"""Trainium2 Bass kernel for nn_DCConv3dKernelPolynomials.

Computes out[m,n,b,p] = sum_k coeff[m,n,k] * psi_k(position[b,p,:])
where psi_k are the 23 real hydrogen-like wavefunctions (n<=4, l<=2).

Key math: with u=x/r, v=y/r, w=z/r, the angular factors are pure
polynomials, so the device only needs sqrt/reciprocal/exp plus
polynomial arithmetic.  All four radial exponentials come from one
exp:  t=e^{-r/12}; e4=t^3, e3=t^4, e2=t^6, e1=t^12.

Perf design (HBM-write-roofline bound, ~94us/core):
  * fp16 output (rel-err ~5e-4, gate is 2e-2) halves the write wall
    vs f32: 32 MiB/core at ~358 GB/s.
  * fp16 K=23 matmul, packed 4-up into 32-row PE strips via
    tile_position=(32i,0): 4 point-blocks computed concurrently, so
    the PE is never the bottleneck (no HAM-throttle sensitivity).
  * poly transposes packed 4-up into 32-col PE strips, written
    straight into a borrowed 4-bank PSUM tile laid out as poly_t4.
  * PSUM->SBUF fp32->fp16 copies on 4-bank [128,2048] tiles (amortizes
    the per-op bubble), split vector/scalar 26/38 to balance engines.
  * per-mt [128,4096] fp16 stages -> 32 x 1 MiB HWDGE DMAs on the
    sync ring (8 KiB/partition contiguous lines).

Sharding: batch b -> core b (8 cores, 4096 points each).
Point order on core: point q lives at (partition p=q%128, chunk
c=q//128), so transposed chunk c lands at poly_t4 block i=c//8
(partitions 32i..32i+22), cols 128*(c%8)..+128 == natural column q.
"""

import math

import numpy as np

B = 8
PTS = 4096            # points per core
OUTC = INC = 64
MN = OUTC * INC       # 4096
NB = 23               # basis functions
NCORES = 8
NCH = 32              # chunks of 128 points
NMT = MN // 128       # 32 mn tiles
NBLK = 4              # point blocks (PE row strips)
PPB = PTS // NBLK     # 1024 points per block
SCALAR_COPIES = 63    # ACT share of the 120 non-forced stage copies


def _combos():
    combos = []
    for n in range(1, 5):
        for k in range(3):
            for m in range(-3, 4):
                if abs(m) <= k and k < n:
                    combos.append((n, k, m))
    return combos


COMBOS = _combos()
assert len(COMBOS) == NB


def _basis_scales():
    """Per-basis constant c_k so that psi_k = c_k * Rb_dev(n,l) * A_dev(l,m).

    Rb_dev / A_dev are the *unnormalized* tile products computed on device:
      Rb_dev(1,0)=e1, Rb_dev(2,0)=(2-r)e2, Rb_dev(2,1)=r*e2,
      Rb_dev(3,0)=(2r-(2/9)r^2-3)e3   [= -L_2^1(2r/3)e3]
      Rb_dev(3,1)=((8/3)r-(4/9)r^2)e3, Rb_dev(3,2)=r^2*e3,
      Rb_dev(4,0)=(4-3r+r^2/2-r^3/48)e4,
      Rb_dev(4,1)=(5r-(5/4)r^2+r^3/16)e4, Rb_dev(4,2)=(6-r/2)r^2*e4
      A_dev: 1, w, u, v, 3w^2-1, wu, wv, u^2-v^2, uv
    """
    fourpi = 4.0 * math.pi
    K00 = math.sqrt(1.0 / fourpi)
    K10 = math.sqrt(3.0 / fourpi)
    K11 = math.sqrt(3.0 / (2.0 * fourpi))
    K20 = math.sqrt(5.0 / fourpi)
    K21 = math.sqrt(5.0 / (6.0 * fourpi))
    K22 = math.sqrt(5.0 / (24.0 * fourpi))
    s2 = math.sqrt(2.0)

    def norm_r(n, l):
        return math.sqrt(
            (2.0 / n) ** 3 * math.factorial(n - l - 1)
            / (2 * n * math.factorial(n + l))
        )

    f = {
        (1, 0): norm_r(1, 0),
        (2, 0): norm_r(2, 0),
        (2, 1): norm_r(2, 1),
        (3, 0): -norm_r(3, 0),
        (3, 1): norm_r(3, 1),
        (3, 2): norm_r(3, 2) * (4.0 / 9.0),
        (4, 0): norm_r(4, 0),
        (4, 1): norm_r(4, 1),
        (4, 2): norm_r(4, 2) * 0.25,
    }
    a = {
        (0, 0): K00,
        (1, 0): K10,
        (1, 1): -s2 * K11,
        (1, -1): -s2 * K11,
        (2, 0): 0.5 * K20,
        (2, 1): -3.0 * s2 * K21,
        (2, -1): -3.0 * s2 * K21,
        (2, 2): 3.0 * s2 * K22,
        (2, -2): 6.0 * s2 * K22,
    }
    return np.array(
        [f[(n, l)] * a[(l, m)] for (n, l, m) in COMBOS], dtype=np.float64
    )


def poly_host(position):
    """Numpy replica of the device basis recipe (for self-checking)."""
    pos = np.asarray(position, dtype=np.float32)
    x, y, z = pos[..., 0], pos[..., 1], pos[..., 2]
    r2 = x * x + y * y + z * z
    r = np.sqrt(r2)
    ir = 1.0 / r
    u, v, w = x * ir, y * ir, z * ir
    e1, e2, e3, e4 = np.exp(-r), np.exp(-r / 2), np.exp(-r / 3), np.exp(-r / 4)
    rr = r * r
    A = {
        (0, 0): np.ones_like(r),
        (1, 0): w, (1, 1): u, (1, -1): v,
        (2, 0): 3 * w * w - 1, (2, 1): w * u, (2, -1): w * v,
        (2, 2): u * u - v * v, (2, -2): u * v,
    }
    Rb = {
        (1, 0): e1,
        (2, 0): (2 - r) * e2,
        (2, 1): r * e2,
        (3, 0): (2 * r - (2.0 / 9.0) * rr - 3) * e3,
        (3, 1): ((8.0 / 3.0) * r - (4.0 / 9.0) * rr) * e3,
        (3, 2): rr * e3,
        (4, 0): (4 - 3 * r + rr / 2 - rr * r / 48) * e4,
        (4, 1): (5 * r - 1.25 * rr + rr * r / 16) * e4,
        (4, 2): (6 - r / 2) * rr * e4,
    }
    c = _basis_scales()
    return np.stack(
        [
            (c[k] * Rb[(n, l)] * A[(l, m)]).astype(np.float32)
            for k, (n, l, m) in enumerate(COMBOS)
        ],
        axis=-1,
    )


_PROGRAM = None


def _build_program():
    import concourse.bacc as bacc
    import concourse.tile as tile
    from concourse import mybir
    from concourse.bass import ts
    from concourse.masks import make_identity

    f32 = mybir.dt.float32
    f16 = mybir.dt.float16
    AF = mybir.ActivationFunctionType
    ALU = mybir.AluOpType

    nc = bacc.Bacc(trn_type="TRN2")
    pos_d = nc.dram_tensor("position", [128, 96], f32, kind="ExternalInput")
    coefft_d = nc.dram_tensor("coefft4", [128, MN], f16, kind="ExternalInput")
    out_d = nc.dram_tensor("out", [MN, PTS], f16, kind="ExternalOutput")

    with tile.TileContext(nc) as tc:
        with (
            tc.tile_pool(name="const", bufs=1) as const,
            tc.tile_pool(name="pw", bufs=1) as pw,
            tc.tile_pool(name="stage", bufs=4) as stage_pool,
            tc.tile_pool(name="psum", bufs=4, space="PSUM") as psum,
        ):
            # inputs on the HWDGE scalar ring (its preamble drains ~2us
            # before the sync ring's; gpsimd is busy with memsets)
            xyz = const.tile([128, 96], f32, tag="xyz", name="xyz")
            nc.scalar.dma_start(out=xyz[:, 0:48], in_=pos_d[:, 0:48])
            nc.scalar.dma_start(out=xyz[:, 48:96], in_=pos_d[:, 48:96])
            coefft4 = const.tile([128, MN], f16, tag="coefft4", name="coefft4_sb")
            nc.scalar.dma_start(out=coefft4[:], in_=coefft_d[:, :])

            ident = const.tile([128, 128], f16, tag="ident", name="ident")
            make_identity(nc, ident[:])

            # preload the exp ACT table on dummy data before the inputs
            # arrive (exp is the ONLY table function used -- r and 1/r come
            # from a DVE bit-trick rsqrt, so no table reloads ever happen)
            warm = pw.tile([128, 16], f32, tag="warm", name="warm")
            warm2 = pw.tile([128, 16], f32, tag="warm2", name="warm2")
            nc.gpsimd.memset(warm[:], 1.0)
            nc.scalar.activation(warm2[:], warm[:], AF.Exp)

            xyz3 = xyz[:].rearrange("p (c t) -> p c t", t=3)

            def T(tag):
                return pw.tile([128, NCH], f32, tag=tag, name=tag)[:]

            def bcastn(ap2d, n):
                import concourse.bass as bass
                return bass.AP(
                    tensor=ap2d.tensor,
                    offset=ap2d.offset,
                    ap=[ap2d.ap[0], [0, n], ap2d.ap[1]],
                )

            # basis values land in poly16[:, c, k] fp16; k padded to 32.
            # Transposing 4 consecutive chunks at once ([128, 4x32] ->
            # [128, 128]) puts chunk 4b+j's psi_k at PSUM rows 32j+k, so
            # strip j receives its own chunk with no replication; the
            # point->(partition, chunk) permutation is folded into the
            # host-side layout (see _prep_inputs).
            poly16p = const.tile([128, NCH, 32], f16, tag="poly16", name="poly16")
            nc.gpsimd.memset(poly16p[:, :, NB:32], 0.0)
            poly_t4 = const.tile([128, PPB], f16, tag="poly_t4", name="poly_t4")

            (r2, r, ir, rr, lnr2, t1, t2, e2, e3, e4, uu, vv, t20, rb21,
             p30, rb31, rb32, p40, p40b, rb41, rb42) = (
                T(t) for t in (
                    "r2 r ir rr lnr2 t1 t2 e2 e3 e4 uu vv t20 rb21 "
                    "p30 rb31 rb32 p40 p40b rb41 rb42"
                ).split()
            )
            vwu_t = pw.tile([128, 3, NCH], f32, tag="vwu", name="vwu")[:]
            ang5_t = pw.tile([128, 5, NCH], f32, tag="ang5", name="ang5")[:]

            def _H(t2d):
                return t2d[:, 0:16]

            def pw_radial(h):
                """r, 1/r, exps, angulars for chunk half h.  Both halves
                share the SAME temp columns so the scheduler is forced to
                finish h0's chain before starting h1 (keeps h0's transposes
                off the back of an interleaved 2x-long vector queue)."""
                sl = slice(16 * h, 16 * (h + 1))
                H = _H

                x, y, z = (xyz3[:, sl, t] for t in range(3))
                hr2, hr, hir, hrr, hnw = (
                    H(t) for t in (r2, r, ir, rr, lnr2)
                )
                nc.vector.tensor_mul(hr2, x, x)
                tAh, tBh = H(uu), H(vv)   # reuse as scratch pre-uu/vv
                nc.vector.tensor_mul(tAh, y, y)
                nc.vector.tensor_add(hr2, hr2, tAh)
                nc.vector.tensor_mul(tBh, z, z)
                nc.vector.tensor_add(hr2, hr2, tBh)
                # 1/r = rsqrt(r2) via the int32 bit trick + 2 Newton steps
                # (rel err ~5e-6, far below the fp16 output rounding)
                i32 = mybir.dt.int32
                nc.vector.tensor_scalar(
                    hir.bitcast(i32), hr2.bitcast(i32), 1, None,
                    ALU.logical_shift_right,
                )
                nc.vector.tensor_scalar(
                    hir.bitcast(i32), hir.bitcast(i32), 0x5F3759DF, -1,
                    ALU.subtract, ALU.mult,
                )
                for _ in range(2):
                    nc.vector.tensor_mul(hnw, hir, hir)
                    nc.vector.tensor_mul(hnw, hnw, hr2)
                    nc.vector.tensor_scalar(
                        hnw, hnw, -0.5, 1.5, ALU.mult, ALU.add
                    )
                    nc.vector.tensor_mul(hir, hir, hnw)
                nc.vector.tensor_mul(hr, hr2, hir)

                # radial exponentials from one exp: t1=e^{-r/12}
                ht1, ht2, he2, he3, he4 = (H(t) for t in (t1, t2, e2, e3, e4))
                nc.scalar.activation(ht1, hr, AF.Exp, scale=-1.0 / 12.0)
                nc.vector.tensor_mul(ht2, ht1, ht1)
                nc.vector.tensor_mul(he4, ht2, ht1)
                nc.vector.tensor_mul(he3, ht2, ht2)
                nc.vector.tensor_mul(he2, he3, ht2)

                vwu = vwu_t[:, :, 0:16]
                ang5 = ang5_t[:, :, 0:16]
                v, w, u = vwu[:, 0, :], vwu[:, 1, :], vwu[:, 2, :]
                uv, wv, a20, wu, a22 = (ang5[:, i, :] for i in range(5))
                nc.vector.tensor_mul(v, y, hir)
                nc.vector.tensor_mul(w, z, hir)
                nc.vector.tensor_mul(u, x, hir)
                nc.vector.tensor_mul(hrr, hr, hr)

                huu, hvv = H(uu), H(vv)
                nc.vector.tensor_mul(a20, w, w)
                nc.vector.tensor_scalar(a20, a20, 3.0, -1.0, ALU.mult, ALU.add)
                nc.vector.tensor_mul(huu, u, u)
                nc.vector.tensor_mul(hvv, v, v)
                nc.vector.tensor_sub(a22, huu, hvv)
                nc.vector.tensor_mul(uv, u, v)
                nc.vector.tensor_mul(wu, w, u)
                nc.vector.tensor_mul(wv, w, v)

            def _poly_views(h):
                sl = slice(16 * h, 16 * (h + 1))
                H = _H
                poly_s = poly16p[:, sl, 0:NB]
                slot = [poly16p[:, sl, k] for k in range(NB)]
                poly_kc = poly_s.rearrange("p c k -> p k c")
                return H, slot, poly_kc

            def pw_poly_a(h):
                """Basis products, slots 0..13."""
                H, slot, poly_kc = _poly_views(h)
                hr, hrr = H(r), H(rr)
                he2, he3 = H(e2), H(e3)
                vwu = vwu_t[:, :, 0:16]
                ang5 = ang5_t[:, :, 0:16]

                nc.vector.tensor_mul(slot[0], he2, he2)          # e1
                ht20 = H(t20)
                nc.vector.tensor_scalar(ht20, hr, -1.0, 2.0, ALU.mult, ALU.add)
                nc.vector.tensor_mul(slot[1], ht20, he2)
                hrb21 = H(rb21)
                nc.vector.tensor_mul(hrb21, hr, he2)
                nc.vector.tensor_mul(poly_kc[:, 2:5, :], bcastn(hrb21, 3), vwu)
                hp30 = H(p30)
                nc.vector.tensor_scalar(hp30, hrr, 2.0 / 9.0, 3.0, ALU.mult, ALU.add)
                nc.vector.scalar_tensor_tensor(
                    hp30, hr, 2.0, hp30, ALU.mult, ALU.subtract
                )
                nc.vector.tensor_mul(slot[5], hp30, he3)
                hrb31 = H(rb31)
                nc.vector.tensor_scalar(
                    hrb31, hr, -4.0 / 9.0, 8.0 / 3.0, ALU.mult, ALU.add
                )
                nc.vector.tensor_mul(hrb31, hrb31, hr)
                nc.vector.tensor_mul(hrb31, hrb31, he3)
                nc.vector.tensor_mul(poly_kc[:, 6:9, :], bcastn(hrb31, 3), vwu)
                hrb32 = H(rb32)
                nc.vector.tensor_mul(hrb32, hrr, he3)
                nc.vector.tensor_mul(poly_kc[:, 9:14, :], bcastn(hrb32, 5), ang5)

            def pw_poly_b(h):
                """Basis products, slots 14..22."""
                H, slot, poly_kc = _poly_views(h)
                hr, hrr = H(r), H(rr)
                he4 = H(e4)
                vwu = vwu_t[:, :, 0:16]
                ang5 = ang5_t[:, :, 0:16]
                hp40, hp40b = H(p40), H(p40b)
                nc.vector.tensor_scalar(hp40, hr, -1.0 / 48.0, 0.5, ALU.mult, ALU.add)
                nc.vector.tensor_mul(hp40, hp40, hrr)
                nc.vector.tensor_scalar(hp40b, hr, -3.0, 4.0, ALU.mult, ALU.add)
                nc.vector.tensor_add(hp40, hp40, hp40b)
                nc.vector.tensor_mul(slot[14], hp40, he4)
                hrb41 = H(rb41)
                nc.vector.tensor_scalar(hrb41, hr, 1.0 / 16.0, -1.25, ALU.mult, ALU.add)
                nc.vector.tensor_mul(hrb41, hrb41, hr)
                nc.vector.tensor_scalar(hrb41, hrb41, 5.0, None, ALU.add)
                nc.vector.tensor_mul(hrb41, hrb41, hr)
                nc.vector.tensor_mul(hrb41, hrb41, he4)
                nc.vector.tensor_mul(poly_kc[:, 15:18, :], bcastn(hrb41, 3), vwu)
                hrb42 = H(rb42)
                nc.vector.tensor_scalar(hrb42, hr, -0.5, 6.0, ALU.mult, ALU.add)
                nc.vector.tensor_mul(hrb42, hrb42, hrr)
                nc.vector.tensor_mul(hrb42, hrb42, he4)
                nc.vector.tensor_mul(poly_kc[:, 18:23, :], bcastn(hrb42, 5), ang5)

            # ---- emission helpers ----------------------------------------
            def transpose_half(h, drain_eng):
                tr = psum.tile([128, 1024], f32, tag="mm", name="tr_ps")
                trv = tr[:].bitcast(f16)
                for bb in range(4):
                    b = 4 * h + bb
                    nc.tensor.transpose(
                        trv[:, 128 * bb:128 * (bb + 1)],
                        poly16p[:, 4 * b:4 * (b + 1), :],
                        ident[:],
                    )
                drain_eng(poly_t4[:, ts(h, 512)], trv[:, 0:512])

            # per (mt, h): 4 strip-concurrent MMs into two 2-bank psum
            # tiles (strips 0,1 -> A; 2,3 -> B), drained by both engines in
            # parallel into stage_ap; psum bank j == stage cols 2048h+512j
            # == points 2048h+512j.. (identity order via the host layout).
            copy_state = [0, 0]   # [idx, scalar_cum]

            def stage_copy(dst, src, paired=None):
                if paired is not None:
                    if paired == 0:
                        nc.scalar.copy(dst, src)
                    else:
                        nc.vector.tensor_copy(dst, src)
                    return
                idx, cum = copy_state
                s_next = ((idx + 1) * SCALAR_COPIES) // 120
                copy_state[0] = idx + 1
                copy_state[1] = s_next
                if s_next > cum:
                    nc.scalar.copy(dst, src)
                else:
                    nc.vector.tensor_copy(dst, src)

            def mm_group(mt, h, stage_ap, paired=False):
                for q in range(2):
                    ps = psum.tile([128, 1024], f32, tag="mm", name="mmps")
                    for jj in range(2):
                        j = 2 * q + jj
                        nc.tensor.matmul(
                            ps[:, ts(jj, 512)],
                            lhsT=coefft4[32 * j:32 * j + NB, ts(mt, 128)],
                            rhs=poly_t4[32 * j:32 * j + NB, ts(h, 512)],
                            start=True,
                            stop=True,
                            tile_position=(32 * j, 0),
                        )
                    stage_copy(
                        stage_ap[:, ts(q, 1024)], ps[:], q if paired else None
                    )

            def half_stage(mt, h, paired=False):
                stage = stage_pool.tile(
                    [128, 2048], f16, tag="stage_h", name="stage_h", bufs=3
                )
                mm_group(mt, h, stage, paired)
                nc.sync.dma_start(
                    out=out_d[ts(mt, 128), ts(h, 2048)], in_=stage[:]
                )

            # ---- emission order: h0 chain; 4 early h0 groups run on the
            # PE between the two transpose batches, as 512 KiB half-stage
            # DMAs; from mt=4 on, full-row 1 MiB stages (8 KiB HBM runs).
            pw_radial(0)
            pw_poly_a(0)
            pw_poly_b(0)
            transpose_half(0, nc.scalar.copy)
            pw_radial(1)
            pw_poly_a(1)
            for mt in range(4):
                half_stage(mt, 0, paired=True)
            pw_poly_b(1)
            transpose_half(1, nc.vector.tensor_copy)
            for mt in range(4):
                half_stage(mt, 1)
            for mt in range(4, NMT - 1):
                stage = stage_pool.tile(
                    [128, PTS], f16, tag="stage", name="stage", bufs=5
                )
                mm_group(mt, 0, stage_ap=stage[:, 0:2048])
                mm_group(mt, 1, stage_ap=stage[:, 2048:4096])
                nc.sync.dma_start(out=out_d[ts(mt, 128), :], in_=stage[:])
            # last mt as two half-stage DMAs -> shorter tail
            half_stage(NMT - 1, 0)
            half_stage(NMT - 1, 1)

    nc.finalize()
    return nc


def _get_program():
    global _PROGRAM
    if _PROGRAM is None:
        _PROGRAM = _build_program()
    return _PROGRAM


def _prep_inputs(position, coefficients):
    pos = np.asarray(position, dtype=np.float32)
    coeff = np.asarray(coefficients, dtype=np.float32)
    assert pos.shape == (B, PTS, 3) and coeff.shape == (OUTC, INC, NB)
    c = _basis_scales().astype(np.float32)
    C = (coeff * c).reshape(MN, NB).T.astype(np.float16)  # [23, 4096]
    coefft4 = np.zeros((128, MN), dtype=np.float16)
    for i in range(NBLK):
        coefft4[32 * i:32 * i + NB] = C
    # point q = 2048h + 512j + 128t + p  ->  (partition p, chunk 16h+4t+j)
    # so that the batch-of-4 transpose + strip-j matmul leaves the output
    # columns in natural point order.
    cs = np.arange(NCH)
    h, t, j = cs // 16, (cs % 16) // 4, cs % 4
    q0 = 2048 * h + 512 * j + 128 * t
    idx = q0[None, :] + np.arange(128)[:, None]   # [128 p, 32 c] -> q
    return [
        {
            "position": np.ascontiguousarray(
                pos[b][idx].reshape(128, 96)
            ),
            "coefft4": coefft4,
        }
        for b in range(B)
    ]


def _assemble(results):
    return np.stack(
        [
            np.asarray(r["out"]).astype(np.float32).reshape(OUTC, INC, PTS)
            for r in results
        ],
        axis=2,
    )


def kernel(position, coefficients):
    from concourse import bass_utils

    nc = _get_program()
    in_maps = _prep_inputs(position, coefficients)
    res = bass_utils.run_bass_kernel_spmd(nc, in_maps, core_ids=list(range(NCORES)))
    return _assemble(res.results)


def kernel_traced(position, coefficients, trace_cores=None):
    """Like kernel() but captures an NTFF trace; returns (out, results)."""
    from concourse import bass_utils

    nc = _get_program()
    in_maps = _prep_inputs(position, coefficients)
    res = bass_utils.run_bass_kernel_spmd(
        nc,
        in_maps,
        core_ids=list(range(NCORES)),
        trace=True,
        trace_cores=trace_cores,
    )
    return _assemble(res.results), res
